# revision 3
# baseline (speedup 1.0000x reference)
"""ActionEncoder Trainium2 kernel (8 NeuronCores, expert-parallel).

Strategy:
- Host groups the 32768 flat actions by action_type (1=pick, 2=transport,
  3=move; type 0 rows are wait_emb and never touch the device), splits each
  group evenly across the 8 cores, and pads each per-core shard to a
  multiple of 128 (pad rows gather table row 0 and are discarded).
- Embedding tables are split on the host into bf16 hi + bf16 lo residual
  (hi+lo ~ fp24, beats fp32r precision). Each core pulls its rows with
  dma_gather(transpose=True) over 4 SWDGE queues, so gathers land directly
  feature-major; a DVE add fuses hi+lo into fp32r activations. Two fp32r
  GEMMs per expert with LeakyReLU(0.01)+bias fused on ScalarE. Output is
  written feature-major [256, C] and un-transposed/scattered on the host.
- Weights/tables are replicated per core; one SPMD NEFF for all 8 cores.
"""
import sys

import numpy as np

sys.path.insert(0, "/opt/trn_rl_repo")

import ml_dtypes

import concourse.bass as bass
import concourse.bacc as bacc
import concourse.mybir as mybir
import concourse.tile as tile
from concourse import library_config
from concourse.bass_utils import run_bass_kernel_spmd


def _ensure_axon_hooks():
    """Some images lack antenv.axon_hooks; register the ctypes NTFF hook
    shim so run_bass_kernel_spmd's trace path works instead of crashing."""
    try:
        import antenv.axon_hooks  # noqa: F401
        return
    except ImportError:
        pass
    import types

    try:
        import antenv
        from trn_agent_boot.trn_boot import _ntff_profile_via_ctypes

        hook = _ntff_profile_via_ctypes("/opt/axon/libaxon_pjrt.so")
    except Exception:
        return
    mod = types.ModuleType("antenv.axon_hooks")
    state = {"hook": hook}
    mod.get_axon_ntff_profile_hook = lambda: state["hook"]
    mod.set_axon_ntff_profile_hook = lambda h: state.update(hook=h)
    sys.modules["antenv.axon_hooks"] = mod
    antenv.axon_hooks = mod


_ensure_axon_hooks()

D = 256
HID = 512
OUT = 256
NTAB = 8192
NCORES = 8
NA = 512  # max actions per compute chunk (matmul moving dim)
FP32 = mybir.dt.float32
FP32R = mybir.dt.float32r
BF16 = mybir.dt.bfloat16
INT16 = mybir.dt.int16

LAST_RESULT = None  # BassKernelResults of the most recent kernel() call

# (name, gathered tables, layer-1 K)
EXPERTS = (
    ("pick", ("agv", "from", "to", "mach"), 4 * D),
    ("trans", ("agv", "mach"), 2 * D),
    ("move", ("agv", "mach"), 2 * D),
)
TABLE_OF = {"agv": "emb_AGV", "from": "emb_operation", "to": "emb_operation", "mach": "emb_machine"}


def _chunks(c):
    """Split capacity c into ~equal chunks of <=NA, multiples of 128."""
    nch = -(-c // NA)
    base = c // nch // 128 * 128
    out = []
    pos = 0
    for i in range(nch):
        n = base if i < nch - 1 else c - pos
        out.append((pos, n))
        pos += n
    return out


def _build(caps):
    """Emit the per-core BIR. caps = dict expert -> padded capacity."""
    nc = bacc.Bacc(num_swdge_queues=4)

    tabs = {}
    for tn in ("emb_operation", "emb_machine", "emb_AGV"):
        tabs[tn] = nc.declare_dram_parameter(f"{tn}_cat", [NTAB, 2 * D], BF16, isOutput=False)

    params = {}
    for name, tables, K in EXPERTS:
        c = caps[name]
        params[f"{name}_W1"] = nc.declare_dram_parameter(f"{name}_W1", [128, K // 128, HID], FP32R, isOutput=False)
        params[f"{name}_W2"] = nc.declare_dram_parameter(f"{name}_W2", [128, HID // 128, OUT], FP32R, isOutput=False)
        params[f"{name}_b1"] = nc.declare_dram_parameter(f"{name}_b1", [128, HID // 128], FP32, isOutput=False)
        params[f"{name}_b2"] = nc.declare_dram_parameter(f"{name}_b2", [128, OUT // 128], FP32, isOutput=False)
        params[f"{name}_outT"] = nc.declare_dram_parameter(f"{name}_outT", [OUT, c], FP32, isOutput=True)

    seg_off = {}
    off = 0
    for name, tables, K in EXPERTS:
        for t in tables:
            seg_off[(name, t)] = off
            off += caps[name] // 16
    params["idx_all"] = nc.declare_dram_parameter("idx_all", [128, off], INT16, isOutput=False)
    params["idx_warm"] = nc.declare_dram_parameter("idx_warm", [128, 8], INT16, isOutput=False)
    params["warm_sum"] = nc.declare_dram_parameter("warm_sum", [128, 1], FP32, isOutput=True)

    qrr = [0]  # SWDGE queue round-robin counter

    with tile.TileContext(nc) as tc:
        with (
            tc.tile_pool(name="wp", bufs=1) as wp,
            tc.tile_pool(name="xp", bufs=2) as xp,
            tc.tile_pool(name="ps", bufs=1, space="PSUM") as ps,
        ):
            nc.gpsimd.load_library(library_config.mlp)

            # --- tiny warm-idx DMA first, warmups open the queues early ---
            idx_warm = wp.tile([128, 8], INT16, name="idx_warm")
            nc.sync.dma_start(out=idx_warm[:], in_=params["idx_warm"][:])
            idx_all = wp.tile([128, off], INT16, name="idx_all")
            nc.sync.dma_start(out=idx_all[:], in_=params["idx_all"][:])
            warms = []
            for q in range(4):
                w = wp.tile([128, 2 * D // 128, 128], BF16, name=f"warm{q}")
                nc.gpsimd.dma_gather(
                    w[:],
                    tabs["emb_operation"][:],
                    idx_warm[:],
                    128,
                    128,
                    2 * D,
                    transpose=True,
                    queue_num=q,
                )
                warms.append(w)
            wsum = wp.tile([128, 1], FP32, name="wsum")
            for q in range(4):
                nc.vector.reduce_sum(
                    wsum[:] if q == 0 else wsum[:],
                    warms[q][:],
                    axis=mybir.AxisListType.XY,
                )
            nc.sync.dma_start(out=params["warm_sum"][:], in_=wsum[:])

            # --- weights, in first-use order ---
            W1 = {}
            W2 = {}
            B1 = {}
            B2 = {}
            for name, tables, K in EXPERTS:
                W1[name] = wp.tile([128, K // 128, HID], FP32R, name=f"w1_{name}")
                nc.sync.dma_start(out=W1[name][:], in_=params[f"{name}_W1"][:])
                B1[name] = wp.tile([128, HID // 128], FP32, name=f"b1_{name}")
                nc.sync.dma_start(out=B1[name][:], in_=params[f"{name}_b1"][:])
                W2[name] = wp.tile([128, HID // 128, OUT], FP32R, name=f"w2_{name}")
                nc.sync.dma_start(out=W2[name][:], in_=params[f"{name}_W2"][:])
                B2[name] = wp.tile([128, OUT // 128], FP32, name=f"b2_{name}")
                nc.sync.dma_start(out=B2[name][:], in_=params[f"{name}_b2"][:])

            # --- compute, chunk by chunk ---
            for name, tables, K in EXPERTS:
                c = caps[name]
                grp = "pick" if name == "pick" else "tm"
                for pos, n in _chunks(c):
                    gh = {}
                    for t in tables:
                        g = wp.tile(
                            [128, 2 * D // 128, n], BF16,
                            name=f"g_{name}_{t}_{pos}",
                        )
                        so = seg_off[(name, t)]
                        nc.gpsimd.dma_gather(
                            g[:],
                            tabs[TABLE_OF[t]][:],
                            idx_all[:, so + pos // 16 : so + (pos + n) // 16],
                            n,
                            n,
                            2 * D,
                            transpose=True,
                            queue_num=qrr[0] % 4,
                        )
                        qrr[0] += 1
                        gh[t] = g

                    # reconstruct feature-major fp32r XT [128, K/128, n]
                    xT = xp.tile([128, K // 128, NA], FP32R, tag=f"xT_{grp}", name=f"xT_{name}")
                    for kd in range(K // 128):
                        t = tables[kd // 2]
                        h = kd % 2
                        nc.vector.tensor_add(
                            out=xT[:, kd, :n],
                            in0=gh[t][:, h, :],
                            in1=gh[t][:, h + 2, :],
                        )

                    # layer 1: H = Prelu(X @ W1 + b1), feature-major
                    hT = xp.tile([128, HID // 128, NA], FP32R, tag="hT", name=f"hT_{name}")
                    for m in range(HID // 128):
                        p1 = ps.tile([128, NA], FP32, space="PSUM", tag="p1", bufs=3, name="p1")
                        for k in range(K // 128):
                            nc.tensor.matmul(
                                out=p1[:, :n],
                                lhsT=W1[name][:, k, m * 128 : (m + 1) * 128],
                                rhs=xT[:, k, :n],
                                start=(k == 0),
                                stop=(k == K // 128 - 1),
                            )
                        nc.scalar.activation(
                            out=hT[:, m, :n],
                            in_=p1[:, :n],
                            func=mybir.ActivationFunctionType.Prelu,
                            bias=B1[name][:, m : m + 1],
                            scale=1.0,
                            alpha=0.01,
                        )

                    # layer 2: O = H @ W2 + b2, feature-major
                    osb = xp.tile([128, OUT // 128, NA], FP32, tag="o", name=f"o_{name}")
                    for m2 in range(OUT // 128):
                        p2 = ps.tile([128, NA], FP32, space="PSUM", tag="p2", bufs=3, name="p2")
                        for k2 in range(HID // 128):
                            nc.tensor.matmul(
                                out=p2[:, :n],
                                lhsT=W2[name][:, k2, m2 * 128 : (m2 + 1) * 128],
                                rhs=hT[:, k2, :n],
                                start=(k2 == 0),
                                stop=(k2 == HID // 128 - 1),
                            )
                        nc.vector.tensor_tensor(
                            out=osb[:, m2, :n],
                            in0=p2[:, :n],
                            in1=B2[name][:, m2 : m2 + 1].to_broadcast([128, n]),
                            op=mybir.AluOpType.add,
                        )
                    for m2 in range(OUT // 128):
                        nc.sync.dma_start(
                            out=params[f"{name}_outT"][m2 * 128 : (m2 + 1) * 128, pos : pos + n],
                            in_=osb[:, m2, :n],
                        )

    nc.finalize()
    return nc


def _wrap_idx(idx, c):
    """int array [c] -> wrapped int16 [128, c//16] for dma_gather."""
    w = idx.astype(np.int16).reshape(c // 16, 16).T
    return np.ascontiguousarray(np.tile(w, (8, 1)))


def _prep_w1(w1):
    """[K, N] -> [128, K//128, N]"""
    k = w1.shape[0]
    return np.ascontiguousarray(w1.reshape(k // 128, 128, -1).transpose(1, 0, 2))


def _prep_b(b):
    """[n] -> [128, n//128]"""
    return np.ascontiguousarray(b.reshape(-1, 128).T)


def kernel(**inputs):
    global LAST_RESULT
    at = np.asarray(inputs["action_type"])
    n_act = at.shape[0]
    out = np.empty((n_act, OUT), dtype=np.float32)

    idx_in = {
        "agv": np.asarray(inputs["agv_idx"]),
        "from": np.asarray(inputs["op_from_idx"]),
        "to": np.asarray(inputs["op_to_idx"]),
        "mach": np.asarray(inputs["machine_idx"]),
    }

    rows = {}
    caps = {}
    pers = {}
    for tcode, (name, tables, K) in zip((1, 2, 3), EXPERTS):
        if tcode == 3:
            r = np.nonzero((at != 0) & (at != 1) & (at != 2))[0]
        else:
            r = np.nonzero(at == tcode)[0]
        rows[name] = r
        pers[name] = -(-max(len(r), 1) // NCORES)  # ceil, >=1
        caps[name] = -(-pers[name] // 128) * 128

    nc = _build(caps)

    # hi/lo bf16 split of the embedding tables (shared across cores)
    tab_split = {}
    for tn in ("emb_operation", "emb_machine", "emb_AGV"):
        t = np.asarray(inputs[tn], dtype=np.float32)
        hi = t.astype(ml_dtypes.bfloat16)
        lo = (t - hi.astype(np.float32)).astype(ml_dtypes.bfloat16)
        tab_split[f"{tn}_cat"] = np.ascontiguousarray(np.concatenate([hi, lo], axis=1))

    in_maps = []
    for core in range(NCORES):
        m = dict(tab_split)
        segs = {}
        for name, tables, K in EXPERTS:
            c = caps[name]
            m[f"{name}_W1"] = _prep_w1(np.asarray(inputs[f"{name}_W1"]))
            m[f"{name}_W2"] = _prep_w1(np.asarray(inputs[f"{name}_W2"]))
            m[f"{name}_b1"] = _prep_b(np.asarray(inputs[f"{name}_b1"]))
            m[f"{name}_b2"] = _prep_b(np.asarray(inputs[f"{name}_b2"]))
            r = rows[name]
            per = pers[name]
            shard = r[core * per : (core + 1) * per]
            pad = np.zeros(c, dtype=np.int64)
            pad[: len(shard)] = shard
            for t in tables:
                segs[(name, t)] = _wrap_idx(idx_in[t][pad], c)
        m["idx_all"] = np.concatenate(
            [segs[(name, t)] for name, tables, K in EXPERTS for t in tables], axis=1
        )
        m["idx_warm"] = np.ascontiguousarray(m["idx_all"][:, :8])
        in_maps.append(m)

    import os

    tmpdir = os.environ.get("BASS_KERNEL_TMPDIR") or None
    res = run_bass_kernel_spmd(nc, in_maps, list(range(NCORES)), tmpdir=tmpdir)
    LAST_RESULT = res

    # assemble
    wait_rows = np.nonzero(at == 0)[0]
    out[wait_rows] = np.asarray(inputs["wait_emb"])[None, :].astype(np.float32)
    for name, tables, K in EXPERTS:
        r = rows[name]
        if len(r) == 0:
            continue
        per = pers[name]
        full = np.concatenate(
            [res.results[core][f"{name}_outT"].T[:per] for core in range(NCORES)],
            axis=0,
        )
        out[r] = full[: len(r)]
    return out



# revision 6
# speedup vs baseline: 1.3230x; 1.3230x over previous
"""ActionEncoder Trainium2 kernel (8 NeuronCores, expert-parallel).

Strategy:
- Host groups the 32768 flat actions by action_type (1=pick, 2=transport,
  3=move; type 0 rows are wait_emb and never touch the device), splits each
  group evenly across the 8 cores, and pads each per-core shard to a
  multiple of 128 (pad rows gather table row 0 and are discarded).
- Everything on-device is bf16 (rel-err budget 2e-2; measured ~2.4e-3):
  embedding tables are cast to bf16 on the host, each core pulls its rows
  with dma_gather(transpose=True) over 4 SWDGE queues so gathers land
  feature-major and feed the matmuls directly (no reconstruct pass).
  All gather preps are issued up-front so GpSimd descriptor generation
  overlaps the MLP compute. Two bf16 GEMMs per expert with
  LeakyReLU(0.01)+bias fused on ScalarE (hidden kept bf16). Output is
  written feature-major [256, C] fp32 and un-transposed/scattered on host.
- Weights/tables are replicated per core; one SPMD NEFF for all 8 cores.
"""
import sys

import numpy as np

sys.path.insert(0, "/opt/trn_rl_repo")

import ml_dtypes

import concourse.bass as bass
import concourse.bacc as bacc
import concourse.mybir as mybir
import concourse.tile as tile
from concourse import library_config
from concourse.bass_utils import run_bass_kernel_spmd


def _ensure_axon_hooks():
    """Some images lack antenv.axon_hooks; register the ctypes NTFF hook
    shim so run_bass_kernel_spmd's trace path works instead of crashing."""
    try:
        import antenv.axon_hooks  # noqa: F401
        return
    except ImportError:
        pass
    import types

    try:
        import antenv
        from trn_agent_boot.trn_boot import _ntff_profile_via_ctypes

        hook = _ntff_profile_via_ctypes("/opt/axon/libaxon_pjrt.so")
    except Exception:
        return
    mod = types.ModuleType("antenv.axon_hooks")
    state = {"hook": hook}
    mod.get_axon_ntff_profile_hook = lambda: state["hook"]
    mod.set_axon_ntff_profile_hook = lambda h: state.update(hook=h)
    sys.modules["antenv.axon_hooks"] = mod
    antenv.axon_hooks = mod


_ensure_axon_hooks()

D = 256
HID = 512
OUT = 256
NTAB = 8192
NCORES = 8
NA = 512  # max actions per compute chunk (matmul moving dim)
FP32 = mybir.dt.float32
BF16 = mybir.dt.bfloat16
INT16 = mybir.dt.int16

LAST_RESULT = None  # BassKernelResults of the most recent kernel() call

# (name, gathered tables, layer-1 K); trans/move first so the first compute
# chunk only waits on 2 gathers instead of pick's 4.
EXPERTS = (
    ("trans", ("agv", "mach"), 2 * D),
    ("move", ("agv", "mach"), 2 * D),
    ("pick", ("agv", "from", "to", "mach"), 4 * D),
)
TABLE_OF = {"agv": "emb_AGV", "from": "emb_operation", "to": "emb_operation", "mach": "emb_machine"}


def _chunks(c):
    """Split capacity c into ~equal chunks of <=NA, multiples of 128."""
    nch = -(-c // NA)
    base = c // nch // 128 * 128
    out = []
    pos = 0
    for i in range(nch):
        n = base if i < nch - 1 else c - pos
        out.append((pos, n))
        pos += n
    return out


def _sched(caps):
    """Interleaved (expert, pos, n) chunk order: round-robin across experts
    so every expert's first chunk is ready early and engine use is smooth."""
    lists = {name: _chunks(caps[name]) for name, _, _ in EXPERTS}
    order = []
    i = 0
    while any(lists.values()):
        for name, _, _ in EXPERTS:
            if lists[name]:
                pos, n = lists[name].pop(0)
                order.append((name, pos, n))
        i += 1
    return order


def _build(caps):
    """Emit the per-core BIR. caps = dict expert -> padded capacity."""
    nc = bacc.Bacc(num_swdge_queues=4)

    tabs = {}
    for tn in ("emb_operation", "emb_machine", "emb_AGV"):
        tabs[tn] = nc.declare_dram_parameter(f"{tn}_b", [NTAB, D], BF16, isOutput=False)

    params = {}
    for name, tables, K in EXPERTS:
        c = caps[name]
        params[f"{name}_W1"] = nc.declare_dram_parameter(f"{name}_W1", [128, K // 128, HID], BF16, isOutput=False)
        params[f"{name}_W2"] = nc.declare_dram_parameter(f"{name}_W2", [128, HID // 128, OUT], BF16, isOutput=False)
        params[f"{name}_b1"] = nc.declare_dram_parameter(f"{name}_b1", [128, HID // 128], FP32, isOutput=False)
        params[f"{name}_b2"] = nc.declare_dram_parameter(f"{name}_b2", [128, OUT // 128], FP32, isOutput=False)
        params[f"{name}_outT"] = nc.declare_dram_parameter(f"{name}_outT", [OUT, c], FP32, isOutput=True)

    seg_off = {}
    off = 0
    for name, tables, K in EXPERTS:
        for t in tables:
            seg_off[(name, t)] = off
            off += caps[name] // 16
    params["idx_all"] = nc.declare_dram_parameter("idx_all", [128, off], INT16, isOutput=False)
    params["idx_warm"] = nc.declare_dram_parameter("idx_warm", [128, 8], INT16, isOutput=False)
    params["warm_sum"] = nc.declare_dram_parameter("warm_sum", [128, 1], FP32, isOutput=True)

    sched = _sched(caps)
    qrr = [0]  # SWDGE queue round-robin counter

    with tile.TileContext(nc) as tc:
        with (
            tc.tile_pool(name="wp", bufs=1) as wp,
            tc.tile_pool(name="xp", bufs=2) as xp,
            tc.tile_pool(name="ps", bufs=1, space="PSUM") as ps,
        ):
            nc.gpsimd.load_library(library_config.mlp)

            # --- tiny warm-idx DMA first, warmups open the queues early ---
            idx_warm = wp.tile([128, 8], INT16, name="idx_warm")
            nc.sync.dma_start(out=idx_warm[:], in_=params["idx_warm"][:])
            idx_all = wp.tile([128, off], INT16, name="idx_all")
            nc.sync.dma_start(out=idx_all[:], in_=params["idx_all"][:])
            warms = []
            for q in range(4):
                w = wp.tile([128, D // 128, 128], BF16, name=f"warm{q}")
                nc.gpsimd.dma_gather(
                    w[:],
                    tabs["emb_operation"][:],
                    idx_warm[:],
                    128,
                    128,
                    D,
                    transpose=True,
                    queue_num=q,
                )
                warms.append(w)

            # --- all main gather preps, in compute order, up-front ---
            gh = {}
            for name, pos, n in sched:
                tables = dict((e[0], e[1]) for e in EXPERTS)[name]
                for t in tables:
                    g = wp.tile(
                        [128, D // 128, n], BF16,
                        name=f"g_{name}_{t}_{pos}",
                    )
                    so = seg_off[(name, t)]
                    nc.gpsimd.dma_gather(
                        g[:],
                        tabs[TABLE_OF[t]][:],
                        idx_all[:, so + pos // 16 : so + (pos + n) // 16],
                        n,
                        n,
                        D,
                        transpose=True,
                        queue_num=qrr[0] % 4,
                    )
                    qrr[0] += 1
                    gh[(name, t, pos)] = g

            # --- weights, in first-use order ---
            W1 = {}
            W2 = {}
            B1 = {}
            B2 = {}
            for name, tables, K in EXPERTS:
                W1[name] = wp.tile([128, K // 128, HID], BF16, name=f"w1_{name}")
                nc.sync.dma_start(out=W1[name][:], in_=params[f"{name}_W1"][:])
                B1[name] = wp.tile([128, HID // 128], FP32, name=f"b1_{name}")
                nc.sync.dma_start(out=B1[name][:], in_=params[f"{name}_b1"][:])
                W2[name] = wp.tile([128, HID // 128, OUT], BF16, name=f"w2_{name}")
                nc.sync.dma_start(out=W2[name][:], in_=params[f"{name}_W2"][:])
                B2[name] = wp.tile([128, OUT // 128], FP32, name=f"b2_{name}")
                nc.sync.dma_start(out=B2[name][:], in_=params[f"{name}_b2"][:])

            # --- compute, chunk by chunk ---
            expert_of = dict((e[0], e) for e in EXPERTS)
            for name, pos, n in sched:
                _, tables, K = expert_of[name]

                # layer 1: H = Prelu(X @ W1 + b1), feature-major; rhs comes
                # straight from the gather tiles (128-feature blocks)
                hT = xp.tile([128, HID // 128, NA], BF16, tag="hT", name=f"hT_{name}")
                for m in range(HID // 128):
                    p1 = ps.tile([128, NA], FP32, space="PSUM", tag="p1", bufs=3, name="p1")
                    for k in range(K // 128):
                        g = gh[(name, tables[k // 2], pos)]
                        nc.tensor.matmul(
                            out=p1[:, :n],
                            lhsT=W1[name][:, k, m * 128 : (m + 1) * 128],
                            rhs=g[:, k % 2, :n],
                            start=(k == 0),
                            stop=(k == K // 128 - 1),
                        )
                    nc.scalar.activation(
                        out=hT[:, m, :n],
                        in_=p1[:, :n],
                        func=mybir.ActivationFunctionType.Prelu,
                        bias=B1[name][:, m : m + 1],
                        scale=1.0,
                        alpha=0.01,
                    )

                # layer 2: O = H @ W2 + b2, feature-major
                osb = xp.tile([128, OUT // 128, NA], FP32, tag="o", name=f"o_{name}")
                for m2 in range(OUT // 128):
                    p2 = ps.tile([128, NA], FP32, space="PSUM", tag="p2", bufs=3, name="p2")
                    for k2 in range(HID // 128):
                        nc.tensor.matmul(
                            out=p2[:, :n],
                            lhsT=W2[name][:, k2, m2 * 128 : (m2 + 1) * 128],
                            rhs=hT[:, k2, :n],
                            start=(k2 == 0),
                            stop=(k2 == HID // 128 - 1),
                        )
                    nc.vector.tensor_tensor(
                        out=osb[:, m2, :n],
                        in0=p2[:, :n],
                        in1=B2[name][:, m2 : m2 + 1].to_broadcast([128, n]),
                        op=mybir.AluOpType.add,
                    )
                for m2 in range(OUT // 128):
                    nc.sync.dma_start(
                        out=params[f"{name}_outT"][m2 * 128 : (m2 + 1) * 128, pos : pos + n],
                        in_=osb[:, m2, :n],
                    )

            # warm-sum consumes the warm gathers so they are live code; at
            # the tail so its sync wait never delays weight/output DMAs
            wsum = wp.tile([128, 1], FP32, name="wsum")
            for q in range(4):
                nc.vector.reduce_sum(
                    wsum[:],
                    warms[q][:],
                    axis=mybir.AxisListType.XY,
                )
            nc.sync.dma_start(out=params["warm_sum"][:], in_=wsum[:])

    nc.finalize()
    return nc


def _wrap_idx(idx, c):
    """int array [c] -> wrapped int16 [128, c//16] for dma_gather."""
    w = idx.astype(np.int16).reshape(c // 16, 16).T
    return np.ascontiguousarray(np.tile(w, (8, 1)))


def _prep_w1(w1):
    """[K, N] -> [128, K//128, N] bf16"""
    k = w1.shape[0]
    return np.ascontiguousarray(
        w1.reshape(k // 128, 128, -1).transpose(1, 0, 2).astype(ml_dtypes.bfloat16)
    )


def _prep_b(b):
    """[n] -> [128, n//128]"""
    return np.ascontiguousarray(b.reshape(-1, 128).T)


def kernel(**inputs):
    global LAST_RESULT
    at = np.asarray(inputs["action_type"])
    n_act = at.shape[0]
    out = np.empty((n_act, OUT), dtype=np.float32)

    idx_in = {
        "agv": np.asarray(inputs["agv_idx"]),
        "from": np.asarray(inputs["op_from_idx"]),
        "to": np.asarray(inputs["op_to_idx"]),
        "mach": np.asarray(inputs["machine_idx"]),
    }

    rows = {}
    caps = {}
    pers = {}
    for tcode, (name, tables, K) in zip((2, 3, 1), EXPERTS):
        if tcode == 3:
            r = np.nonzero((at != 0) & (at != 1) & (at != 2))[0]
        else:
            r = np.nonzero(at == tcode)[0]
        rows[name] = r
        pers[name] = -(-max(len(r), 1) // NCORES)  # ceil, >=1
        caps[name] = -(-pers[name] // 128) * 128

    nc = _build(caps)

    # bf16 cast of the embedding tables (shared across cores)
    tab_b = {}
    for tn in ("emb_operation", "emb_machine", "emb_AGV"):
        t = np.asarray(inputs[tn], dtype=np.float32)
        tab_b[f"{tn}_b"] = np.ascontiguousarray(t.astype(ml_dtypes.bfloat16))

    in_maps = []
    for core in range(NCORES):
        m = dict(tab_b)
        segs = {}
        for name, tables, K in EXPERTS:
            c = caps[name]
            m[f"{name}_W1"] = _prep_w1(np.asarray(inputs[f"{name}_W1"]))
            m[f"{name}_W2"] = _prep_w1(np.asarray(inputs[f"{name}_W2"]))
            m[f"{name}_b1"] = _prep_b(np.asarray(inputs[f"{name}_b1"]))
            m[f"{name}_b2"] = _prep_b(np.asarray(inputs[f"{name}_b2"]))
            r = rows[name]
            per = pers[name]
            shard = r[core * per : (core + 1) * per]
            pad = np.zeros(c, dtype=np.int64)
            pad[: len(shard)] = shard
            for t in tables:
                segs[(name, t)] = _wrap_idx(idx_in[t][pad], c)
        m["idx_all"] = np.concatenate(
            [segs[(name, t)] for name, tables, K in EXPERTS for t in tables], axis=1
        )
        m["idx_warm"] = np.ascontiguousarray(m["idx_all"][:, :8])
        in_maps.append(m)

    import os

    tmpdir = os.environ.get("BASS_KERNEL_TMPDIR") or None
    res = run_bass_kernel_spmd(nc, in_maps, list(range(NCORES)), tmpdir=tmpdir)
    LAST_RESULT = res

    # assemble
    wait_rows = np.nonzero(at == 0)[0]
    out[wait_rows] = np.asarray(inputs["wait_emb"])[None, :].astype(np.float32)
    for name, tables, K in EXPERTS:
        r = rows[name]
        if len(r) == 0:
            continue
        per = pers[name]
        full = np.concatenate(
            [res.results[core][f"{name}_outT"].T[:per] for core in range(NCORES)],
            axis=0,
        )
        out[r] = full[: len(r)]
    return out


# revision 8
# speedup vs baseline: 1.4776x; 1.1169x over previous
"""ActionEncoder Trainium2 kernel (8 NeuronCores, expert-parallel).

Strategy:
- Host groups the 32768 flat actions by action_type (1=pick, 2=transport,
  3=move; type 0 rows are wait_emb and never touch the device), splits each
  group evenly across the 8 cores, and pads each per-core shard to a
  multiple of 128 (pad rows use table row 0 and are discarded).
- Everything on-device is bf16 (rel-err budget 2e-2; measured ~2.4e-3).
- Startup: the GPSIMD ucode library reload gates SWDGE gathers for the
  first ~15us of the kernel, so the host pre-gathers each expert's FIRST
  chunk into a dense feature-major xT param that streams in over the
  hardware DGE at full bandwidth; the MLPs start on those while the
  GPSIMD farm preps dma_gather descriptors for the remaining chunks
  (<=8 gathers so the 8 SWDGE completion semaphores never recycle).
- Two bf16 GEMMs per expert with LeakyReLU(0.01)+bias fused on ScalarE
  (hidden stays bf16). Output is written feature-major [256, C] fp32 and
  un-transposed/scattered on the host.
- Weights/tables are replicated per core; W1+W2 are packed into one bf16
  param per expert and all biases into one fp32 param (fewer, larger DGE
  descriptors). One SPMD NEFF for all 8 cores.
"""
import sys

import numpy as np

sys.path.insert(0, "/opt/trn_rl_repo")

import ml_dtypes

import concourse.bass as bass
import concourse.bacc as bacc
import concourse.mybir as mybir
import concourse.tile as tile
from concourse import library_config
from concourse.bass_utils import run_bass_kernel_spmd


def _ensure_axon_hooks():
    """Some images lack antenv.axon_hooks; register the ctypes NTFF hook
    shim so run_bass_kernel_spmd's trace path works instead of crashing."""
    try:
        import antenv.axon_hooks  # noqa: F401
        return
    except ImportError:
        pass
    import types

    try:
        import antenv
        from trn_agent_boot.trn_boot import _ntff_profile_via_ctypes

        hook = _ntff_profile_via_ctypes("/opt/axon/libaxon_pjrt.so")
    except Exception:
        return
    mod = types.ModuleType("antenv.axon_hooks")
    state = {"hook": hook}
    mod.get_axon_ntff_profile_hook = lambda: state["hook"]
    mod.set_axon_ntff_profile_hook = lambda h: state.update(hook=h)
    sys.modules["antenv.axon_hooks"] = mod
    antenv.axon_hooks = mod


_ensure_axon_hooks()

D = 256
HID = 512
OUT = 256
NTAB = 8192
NCORES = 8
NA = 512  # max actions per compute chunk (matmul moving dim)
FP32 = mybir.dt.float32
BF16 = mybir.dt.bfloat16
INT16 = mybir.dt.int16

LAST_RESULT = None  # BassKernelResults of the most recent kernel() call

# (name, gathered tables, layer-1 K); trans/move first so the first
# pre-gathered chunks are the cheap ones and pick's device gathers have
# the longest runway.
EXPERTS = (
    ("trans", ("agv", "mach"), 2 * D),
    ("move", ("agv", "mach"), 2 * D),
    ("pick", ("agv", "from", "to", "mach"), 4 * D),
)
TABLE_OF = {"agv": "emb_AGV", "from": "emb_operation", "to": "emb_operation", "mach": "emb_machine"}


def _dev_chunks(c):
    """Device-gathered chunks of (pos, n) covering [NA, c): <=NA each,
    multiples of 128. Chunk 0 ([0, min(NA, c))) is host-pre-gathered."""
    rem = c - min(NA, c)
    if rem == 0:
        return []
    nch = -(-rem // NA)
    base = rem // nch // 128 * 128
    out = []
    pos = NA
    left = rem
    for i in range(nch):
        n = base if i < nch - 1 else left
        out.append((pos, n))
        pos += n
        left -= n
    return out


def _sched(caps):
    """Interleaved (expert, pos, n, is_dev) chunk order, round-robin across
    experts: every expert's chunk 0 first, then device chunks."""
    lists = {}
    for name, _, _ in EXPERTS:
        c = caps[name]
        lists[name] = [(0, min(NA, c), False)] + [(p, n, True) for p, n in _dev_chunks(c)]
    order = []
    while any(lists.values()):
        for name, _, _ in EXPERTS:
            if lists[name]:
                pos, n, dev = lists[name].pop(0)
                order.append((name, pos, n, dev))
    return order


def _build(caps):
    """Emit the per-core BIR. caps = dict expert -> padded capacity."""
    nc = bacc.Bacc(num_swdge_queues=4)

    tabs = {}
    for tn in ("emb_operation", "emb_machine", "emb_AGV"):
        tabs[tn] = nc.declare_dram_parameter(f"{tn}_b", [NTAB, D], BF16, isOutput=False)

    params = {}
    w1sz = {}
    for name, tables, K in EXPERTS:
        c = caps[name]
        w1sz[name] = (K // 128) * HID
        wsz = w1sz[name] + (HID // 128) * OUT
        params[f"{name}_w"] = nc.declare_dram_parameter(f"{name}_w", [128, wsz], BF16, isOutput=False)
        params[f"{name}_x0"] = nc.declare_dram_parameter(
            f"{name}_x0", [128, K // 128, min(NA, c)], BF16, isOutput=False
        )
        params[f"{name}_outT"] = nc.declare_dram_parameter(f"{name}_outT", [OUT, c], FP32, isOutput=True)
    # biases: per expert [b1 (HID//128) | b2 (OUT//128)] fp32 columns
    nb = HID // 128 + OUT // 128
    params["biases"] = nc.declare_dram_parameter("biases", [128, nb * len(EXPERTS)], FP32, isOutput=False)

    sched = _sched(caps)
    dev_sched = [s for s in sched if s[3]]

    # wrapped int16 index segments, only for device-gathered rows [NA, c)
    seg_off = {}
    off = 0
    for name, tables, K in EXPERTS:
        dev_rows = caps[name] - min(NA, caps[name])
        for t in tables:
            seg_off[(name, t)] = off
            off += dev_rows // 16
    if off:
        params["idx_all"] = nc.declare_dram_parameter("idx_all", [128, off], INT16, isOutput=False)

    qrr = [0]  # SWDGE queue round-robin counter

    with tile.TileContext(nc) as tc:
        with (
            tc.tile_pool(name="wp", bufs=1) as wp,
            tc.tile_pool(name="xp", bufs=2) as xp,
            tc.tile_pool(name="ps", bufs=1, space="PSUM") as ps,
        ):
            if dev_sched:
                nc.gpsimd.load_library(library_config.mlp)
                idx_all = wp.tile([128, off], INT16, name="idx_all")
                nc.sync.dma_start(out=idx_all[:], in_=params["idx_all"][:])

            # --- device gather preps, issued up-front in compute order ---
            gh = {}
            for name, pos, n, _ in dev_sched:
                tables = dict((e[0], e[1]) for e in EXPERTS)[name]
                base = min(NA, caps[name])
                for t in tables:
                    g = wp.tile([128, D // 128, n], BF16, name=f"g_{name}_{t}_{pos}")
                    so = seg_off[(name, t)]
                    nc.gpsimd.dma_gather(
                        g[:],
                        tabs[TABLE_OF[t]][:],
                        idx_all[:, so + (pos - base) // 16 : so + (pos + n - base) // 16],
                        n,
                        n,
                        D,
                        transpose=True,
                        queue_num=qrr[0] % 4,
                    )
                    qrr[0] += 1
                    gh[(name, t, pos)] = g

            # --- pre-gathered first chunks + weights, in first-use order:
            # x0+weights for expert 1 land before expert 2's start, so the
            # first MLP begins as early as possible ---
            X0 = {}
            W = {}
            BIA = wp.tile([128, nb * len(EXPERTS)], FP32, name="biases")
            for i, (name, tables, K) in enumerate(EXPERTS):
                X0[name] = wp.tile([128, K // 128, min(NA, caps[name])], BF16, name=f"x0_{name}")
                nc.sync.dma_start(out=X0[name][:], in_=params[f"{name}_x0"][:])
                if i == 0:
                    nc.sync.dma_start(out=BIA[:], in_=params["biases"][:])
                wsz = w1sz[name] + (HID // 128) * OUT
                W[name] = wp.tile([128, wsz], BF16, name=f"w_{name}")
                nc.sync.dma_start(out=W[name][:], in_=params[f"{name}_w"][:])

            # --- compute, chunk by chunk ---
            expert_of = dict((e[0], e) for e in EXPERTS)
            eidx = dict((e[0], i) for i, e in enumerate(EXPERTS))
            for name, pos, n, dev in sched:
                _, tables, K = expert_of[name]
                boff = eidx[name] * nb

                def rhs1(k):
                    if dev:
                        return gh[(name, tables[k // 2], pos)][:, k % 2, :n]
                    return X0[name][:, k, :n]

                # layer 1: H = Prelu(X @ W1 + b1), feature-major
                hT = xp.tile([128, HID // 128, NA], BF16, tag="hT", name=f"hT_{name}")
                for m in range(HID // 128):
                    p1 = ps.tile([128, NA], FP32, space="PSUM", tag="p1", bufs=3, name="p1")
                    for k in range(K // 128):
                        nc.tensor.matmul(
                            out=p1[:, :n],
                            lhsT=W[name][:, k * HID + m * 128 : k * HID + (m + 1) * 128],
                            rhs=rhs1(k),
                            start=(k == 0),
                            stop=(k == K // 128 - 1),
                        )
                    nc.scalar.activation(
                        out=hT[:, m, :n],
                        in_=p1[:, :n],
                        func=mybir.ActivationFunctionType.Prelu,
                        bias=BIA[:, boff + m : boff + m + 1],
                        scale=1.0,
                        alpha=0.01,
                    )

                # layer 2: O = H @ W2 + b2, feature-major
                osb = xp.tile([128, OUT // 128, NA], FP32, tag="o", name=f"o_{name}")
                for m2 in range(OUT // 128):
                    p2 = ps.tile([128, NA], FP32, space="PSUM", tag="p2", bufs=3, name="p2")
                    for k2 in range(HID // 128):
                        nc.tensor.matmul(
                            out=p2[:, :n],
                            lhsT=W[name][
                                :,
                                w1sz[name] + k2 * OUT + m2 * 128 : w1sz[name] + k2 * OUT + (m2 + 1) * 128,
                            ],
                            rhs=hT[:, k2, :n],
                            start=(k2 == 0),
                            stop=(k2 == HID // 128 - 1),
                        )
                    nc.vector.tensor_tensor(
                        out=osb[:, m2, :n],
                        in0=p2[:, :n],
                        in1=BIA[:, boff + HID // 128 + m2 : boff + HID // 128 + m2 + 1].to_broadcast([128, n]),
                        op=mybir.AluOpType.add,
                    )
                for m2 in range(OUT // 128):
                    nc.sync.dma_start(
                        out=params[f"{name}_outT"][m2 * 128 : (m2 + 1) * 128, pos : pos + n],
                        in_=osb[:, m2, :n],
                    )

    nc.finalize()
    return nc


def _wrap_idx(idx):
    """int array [c] -> wrapped int16 [128, c//16] for dma_gather."""
    c = len(idx)
    w = idx.astype(np.int16).reshape(c // 16, 16).T
    return np.ascontiguousarray(np.tile(w, (8, 1)))


def _pack_w(w1, w2):
    """W1 [K, HID], W2 [HID, OUT] -> [128, K//128*HID + HID//128*OUT] bf16"""
    k = w1.shape[0]
    a = w1.reshape(k // 128, 128, HID).transpose(1, 0, 2).reshape(128, -1)
    b = w2.reshape(HID // 128, 128, OUT).transpose(1, 0, 2).reshape(128, -1)
    return np.ascontiguousarray(np.concatenate([a, b], axis=1).astype(ml_dtypes.bfloat16))


def _prep_b(b):
    """[n] -> [128, n//128]"""
    return np.ascontiguousarray(b.reshape(-1, 128).T)


def kernel(**inputs):
    global LAST_RESULT
    at = np.asarray(inputs["action_type"])
    n_act = at.shape[0]
    out = np.empty((n_act, OUT), dtype=np.float32)

    idx_in = {
        "agv": np.asarray(inputs["agv_idx"]),
        "from": np.asarray(inputs["op_from_idx"]),
        "to": np.asarray(inputs["op_to_idx"]),
        "mach": np.asarray(inputs["machine_idx"]),
    }

    rows = {}
    caps = {}
    pers = {}
    for tcode, (name, tables, K) in zip((2, 3, 1), EXPERTS):
        if tcode == 3:
            r = np.nonzero((at != 0) & (at != 1) & (at != 2))[0]
        else:
            r = np.nonzero(at == tcode)[0]
        rows[name] = r
        pers[name] = -(-max(len(r), 1) // NCORES)  # ceil, >=1
        caps[name] = -(-pers[name] // 128) * 128

    nc = _build(caps)

    # bf16 cast of the embedding tables (shared across cores)
    tab_b = {}
    for tn in ("emb_operation", "emb_machine", "emb_AGV"):
        t = np.asarray(inputs[tn], dtype=np.float32)
        tab_b[f"{tn}_b"] = np.ascontiguousarray(t.astype(ml_dtypes.bfloat16))

    wpk = {}
    for name, tables, K in EXPERTS:
        wpk[f"{name}_w"] = _pack_w(
            np.asarray(inputs[f"{name}_W1"]), np.asarray(inputs[f"{name}_W2"])
        )
    bias = np.concatenate(
        [
            np.concatenate(
                [_prep_b(np.asarray(inputs[f"{name}_b1"])), _prep_b(np.asarray(inputs[f"{name}_b2"]))],
                axis=1,
            )
            for name, tables, K in EXPERTS
        ],
        axis=1,
    ).astype(np.float32)

    in_maps = []
    for core in range(NCORES):
        m = dict(tab_b)
        m.update(wpk)
        m["biases"] = np.ascontiguousarray(bias)
        segs = []
        for name, tables, K in EXPERTS:
            c = caps[name]
            base = min(NA, c)
            r = rows[name]
            per = pers[name]
            shard = r[core * per : (core + 1) * per]
            pad = np.zeros(c, dtype=np.int64)
            pad[: len(shard)] = shard
            # host pre-gather of chunk 0 -> dense feature-major xT
            x0 = np.empty((128, K // 128, base), dtype=ml_dtypes.bfloat16)
            for ti, t in enumerate(tables):
                g = tab_b[f"{TABLE_OF[t]}_b"][idx_in[t][pad[:base]]]  # [base, D] bf16
                gt = g.T.reshape(D // 128, 128, base)  # [2, 128, base]
                x0[:, 2 * ti, :] = gt[0]
                x0[:, 2 * ti + 1, :] = gt[1]
            m[f"{name}_x0"] = np.ascontiguousarray(x0)
            for t in tables:
                if c > base:
                    segs.append(_wrap_idx(idx_in[t][pad[base:]]))
        if segs:
            m["idx_all"] = np.concatenate(segs, axis=1)
        in_maps.append(m)

    import os

    tmpdir = os.environ.get("BASS_KERNEL_TMPDIR") or None
    res = run_bass_kernel_spmd(nc, in_maps, list(range(NCORES)), tmpdir=tmpdir)
    LAST_RESULT = res

    # assemble
    wait_rows = np.nonzero(at == 0)[0]
    out[wait_rows] = np.asarray(inputs["wait_emb"])[None, :].astype(np.float32)
    for name, tables, K in EXPERTS:
        r = rows[name]
        if len(r) == 0:
            continue
        per = pers[name]
        full = np.concatenate(
            [res.results[core][f"{name}_outT"].T[:per] for core in range(NCORES)],
            axis=0,
        )
        out[r] = full[: len(r)]
    return out
